# revision 1
# baseline (speedup 1.0000x reference)
"""Longformer self-attention (BART-style) Trainium2 kernel.

Sharding: 8 cores = 2 batches x 4 head-groups (4 heads each).  Each core:
  - projects q/k/v/kg/vg (+qg for the G global query rows) for its 4 heads
    from a host-pre-transposed xT [E, S] bf16,
  - computes window attention (w=256 band + G=64 global keys, joint softmax)
    in scores-transposed layout [keys, q] so no on-device transposes are
    needed anywhere,
  - computes full attention for the G global query rows with the g-projections,
  - produces ctxT [256 feats, S] bf16; a 4-core AllGather assembles the full
    ctxT [1024, S], and each core then computes a 256-wide E-column slice of
    the output projection (rank-dependence comes only from per-core inputs).

Host folds: q-scale 1/8 into Wq/bq (and Wqg/bqg), v/vg biases into the output
projection bias (window rows use bv-fold, global rows bvg-fold).  Softmax is
computed without max-subtraction (scores are O(1) here) via an appended
ones-column on v giving the denominator for free.
"""
import sys
import numpy as np

sys.path.insert(0, "/opt/trn_rl_repo")

import ml_dtypes

import concourse.bass as bass
import concourse.bacc as bacc
import concourse.tile as tile
from concourse import mybir
from concourse.bass_utils import run_bass_kernel_spmd

BF16 = ml_dtypes.bfloat16
B, S, E, H, D, W, G = 2, 4096, 1024, 16, 64, 256, 64
QB = 256           # query block for window attention
NKC = S // 128     # 32 key chunks
NQB = S // QB      # 16 query blocks
dt = mybir.dt
AF = mybir.ActivationFunctionType


def _build_bass():
    nc = bacc.Bacc("TRN2", num_devices=8)

    def inp(name, shape, dtype=dt.bfloat16):
        return nc.declare_dram_parameter(name, list(shape), dtype, isOutput=False)

    xt = inp("xt", [E, S])                      # x[b].T, bf16
    wq = inp("wq", [E, 256])                    # pre-scaled by 1/8
    wk = inp("wk", [E, 256])
    wv = inp("wv", [E, 256])
    wkg = inp("wkg", [E, 256])
    wvg = inp("wvg", [E, 256])
    wqg = inp("wqg", [E, 256])                  # pre-scaled by 1/8
    wo = inp("wo", [E, 256])                    # E-column slice of Wo
    bq = inp("bq", [256], dt.float32)           # pre-scaled by 1/8
    bk = inp("bk", [256], dt.float32)
    bkg = inp("bkg", [256], dt.float32)
    bqg = inp("bqg", [256], dt.float32)         # pre-scaled by 1/8
    bow = inp("bow", [256], dt.float32)         # bo + bv@Wo   (col slice)
    bog = inp("bog", [256], dt.float32)         # bo + bvg@Wo  (col slice)
    masks = inp("masks", [4, 128, QB])          # bf16 0/1 band masks
    outt = nc.declare_dram_parameter("outt", [2, 128, S], dt.float32, isOutput=True)

    with tile.TileContext(nc) as tc:
        _emit(tc, xt, wq, wk, wv, wkg, wvg, wqg, wo, bq, bk, bkg, bqg,
              bow, bog, masks, outt)
    nc.compile()
    return nc


def _emit(tc, xt, wq, wk, wv, wkg, wvg, wqg, wo, bq, bk, bkg, bqg,
          bow, bog, masks, outt):
    nc = tc.nc
    import contextlib
    ctx = contextlib.ExitStack()

    persist = ctx.enter_context(tc.tile_pool(name="persist", bufs=1))
    xchunk = ctx.enter_context(tc.tile_pool(name="xchunk", bufs=2))
    ptiles = ctx.enter_context(tc.tile_pool(name="ptiles", bufs=3))
    gtiles = ctx.enter_context(tc.tile_pool(name="gtiles", bufs=2))
    small = ctx.enter_context(tc.tile_pool(name="small", bufs=2))
    rbpool = ctx.enter_context(tc.tile_pool(name="rbpool", bufs=3))
    otile = ctx.enter_context(tc.tile_pool(name="otile", bufs=2))
    dram = ctx.enter_context(tc.tile_pool(name="dram", bufs=2, space="DRAM"))
    ps_big = ctx.enter_context(tc.tile_pool(name="ps_big", bufs=2, space="PSUM"))
    ps_s = ctx.enter_context(tc.tile_pool(name="ps_s", bufs=4, space="PSUM"))
    ps_o = ctx.enter_context(tc.tile_pool(name="ps_o", bufs=2, space="PSUM"))

    # ---- load weights / biases / masks ----------------------------------
    def ld_w(t):
        sb = persist.tile([128, 8, 256], dt.bfloat16, name=t.name + "_sb")
        nc.sync.dma_start(out=sb, in_=t[:].rearrange("(kt p) m -> p kt m", p=128))
        return sb

    wq_sb, wk_sb, wv_sb = ld_w(wq), ld_w(wk), ld_w(wv)
    wkg_sb, wvg_sb, wqg_sb, wo_sb = ld_w(wkg), ld_w(wvg), ld_w(wqg), ld_w(wo)

    def ld_b(t):
        sb = persist.tile([128, 2], dt.float32, name=t.name + "_sb")
        nc.sync.dma_start(out=sb, in_=t[:].rearrange("(hp p) -> p hp", p=128))
        return sb

    bq_sb, bk_sb, bkg_sb, bqg_sb = ld_b(bq), ld_b(bk), ld_b(bkg), ld_b(bqg)
    bow_sb, bog_sb = ld_b(bow), ld_b(bog)

    mask_sb = persist.tile([128, 4, QB], dt.bfloat16, name="mask_sb")
    nc.sync.dma_start(out=mask_sb, in_=masks[:].rearrange("m p q -> p m q"))

    # ---- persistent activation tiles ------------------------------------
    qT_sb = persist.tile([128, 2, S], dt.bfloat16, name="qT")
    kT_sb = persist.tile([128, 2, S], dt.bfloat16, name="kT")
    kgT_sb = persist.tile([128, 2, S], dt.bfloat16, name="kgT")
    qgT_sb = persist.tile([128, 2, G], dt.bfloat16, name="qgT")
    v_sb = persist.tile([128, 32, 4, 65], dt.bfloat16, name="v")
    vg_sb = persist.tile([128, 32, 4, 65], dt.bfloat16, name="vg")
    ctxT_sb = persist.tile([128, 2, S], dt.bfloat16, name="ctxT")

    nc.vector.memset(v_sb[:, :, :, 64:65], 1.0)
    nc.vector.memset(vg_sb[:, :, :, 64:65], 1.0)

    # ---- projections, streamed over 512-col chunks of xT ----------------
    for ncnk in range(8):
        cs = slice(ncnk * 512, ncnk * 512 + 512)
        xc = xchunk.tile([128, 8, 512], dt.bfloat16, tag="xc", name="xc")
        nc.sync.dma_start(
            out=xc, in_=xt[:, cs].rearrange("(kt p) s -> p kt s", p=128))

        # transposed projections: psum[dpair, s] = W.T @ x
        for w_sb, b_sb, dst in ((wq_sb, bq_sb, qT_sb), (wk_sb, bk_sb, kT_sb),
                                (wkg_sb, bkg_sb, kgT_sb)):
            for hp in range(2):
                ps = ps_big.tile([128, 512], dt.float32, tag="ps_big", name="ps_big")
                for kt in range(8):
                    nc.tensor.matmul(
                        ps, w_sb[:, kt, hp * 128:hp * 128 + 128],
                        xc[:, kt, :], start=(kt == 0), stop=(kt == 7))
                nc.vector.tensor_scalar_add(
                    dst[:, hp, cs], ps, b_sb[:, hp:hp + 1])

        # natural-layout v / vg (no bias; ones column already set)
        for w_sb, dst in ((wv_sb, v_sb), (wvg_sb, vg_sb)):
            for sc in range(4):                 # four 128-row chunks
                s32 = ncnk * 4 + sc
                ps = ps_big.tile([128, 256], dt.float32, tag="ps_big", name="ps_big")
                for kt in range(8):
                    nc.tensor.matmul(
                        ps, xc[:, kt, sc * 128:sc * 128 + 128],
                        w_sb[:, kt, :], start=(kt == 0), stop=(kt == 7))
                nc.vector.tensor_copy(
                    out=dst[:, s32, :, 0:64],
                    in_=ps[:].rearrange("p (h d) -> p h d", h=4))

        if ncnk == 0:                           # qgT from the first G columns
            for hp in range(2):
                ps = ps_big.tile([128, G], dt.float32, tag="ps_big", name="ps_big")
                for kt in range(8):
                    nc.tensor.matmul(
                        ps, wqg_sb[:, kt, hp * 128:hp * 128 + 128],
                        xc[:, kt, 0:G], start=(kt == 0), stop=(kt == 7))
                nc.vector.tensor_scalar_add(
                    qgT_sb[:, hp, :], ps, bqg_sb[:, hp:hp + 1])

    # ---- global-query attention (full attention, g-projections) ---------
    dens_g = small.tile([4, G], dt.float32, tag="dens_g", name="dens_g")
    for hp in range(2):
        for hi2 in range(2):
            hi = hp * 2 + hi2
            row = hi2 * 64
            pgT = gtiles.tile([128, NKC, G], dt.bfloat16, tag="pgT", name="pgT")
            for kc4 in range(NKC // 4):
                pss = ps_s.tile([128, 4, G], dt.float32, tag="ps_s",
                                name="ps_s")
                for t in range(4):
                    kc = kc4 * 4 + t
                    nc.tensor.matmul(
                        pss[:, t, :],
                        kgT_sb[row:row + 64, hp, kc * 128:kc * 128 + 128],
                        qgT_sb[row:row + 64, hp, :], start=True, stop=True)
                nc.scalar.activation(pgT[:, kc4 * 4:kc4 * 4 + 4, :], pss,
                                     AF.Exp)
            pv = ps_o.tile([128, QB], dt.float32, tag="ps_o", name="ps_o")[:65, :G]
            for kc in range(NKC):
                nc.tensor.matmul(
                    pv, vg_sb[:, kc, hi, :], pgT[:, kc, :],
                    start=(kc == 0), stop=(kc == NKC - 1))
            nc.vector.tensor_copy(ctxT_sb[row:row + 64, hp, 0:G], pv[0:64, :])
            d64 = rbpool.tile([128, QB], dt.float32, tag="rb", name="rb")
            nc.vector.tensor_copy(d64[64:65, :G], pv[64:65, :])
            nc.sync.dma_start(out=dens_g[hi:hi + 1, :], in_=d64[64:65, :G])
    recg = small.tile([4, G], dt.float32, tag="recg", name="recg")
    nc.vector.reciprocal(recg, dens_g)
    drecg = dram.tile([4, G], dt.float32, tag="drecg", name="drecg")
    nc.sync.dma_start(out=drecg, in_=recg)
    for hi in range(4):
        hp, row = hi // 2, (hi % 2) * 64
        rbg = rbpool.tile([128, QB], dt.float32, tag="rb", name="rb")
        rbg = rbg[row:row + 64, :G]
        src_ap = drecg[hi:hi + 1, :]
        src_ap = bass.AP(tensor=src_ap.tensor, offset=src_ap.offset,
                         ap=[[0, 64]] + list(src_ap.ap[1:]))
        nc.gpsimd.dma_start(out=rbg, in_=src_ap)
        nc.vector.tensor_mul(ctxT_sb[row:row + 64, hp, 0:G],
                             ctxT_sb[row:row + 64, hp, 0:G], rbg)

    # ---- window attention + gather + output projection ------------------
    for scnk in range(8):                       # 512-column output chunks
        dens = small.tile([8, QB], dt.float32, tag="dens", name="dens")
        for qb in range(scnk * 2, scnk * 2 + 2):
            q0 = qb * QB
            ql = qb % 2
            base = q0 // 128
            kcs = [base + d for d in range(-2, 4) if 0 <= base + d < NKC]
            for hp in range(2):
                for hi2 in range(2):
                    hi = hp * 2 + hi2
                    row = hi2 * 64
                    pT = ptiles.tile([128, 7, QB], dt.bfloat16, tag="pT", name="pT")
                    # global keys first (standard projections, rows 0:G)
                    psg = ps_s.tile([128, 2, QB], dt.float32, tag="ps_s",
                                    name="ps_s")
                    nc.tensor.matmul(
                        psg[0:G, 0, :], kT_sb[row:row + 64, hp, 0:G],
                        qT_sb[row:row + 64, hp, q0:q0 + QB],
                        start=True, stop=True)
                    nc.scalar.activation(pT[0:G, 6, :], psg[0:G, 0, :], AF.Exp)
                    for gi in range(0, len(kcs), 2):
                        grp = kcs[gi:gi + 2]
                        pss = ps_s.tile([128, 2, QB], dt.float32, tag="ps_s",
                                        name="ps_s")
                        for t, kc in enumerate(grp):
                            nc.tensor.matmul(
                                pss[:, t, :],
                                kT_sb[row:row + 64, hp, kc * 128:kc * 128 + 128],
                                qT_sb[row:row + 64, hp, q0:q0 + QB],
                                start=True, stop=True)
                        nc.scalar.activation(
                            pT[:, gi:gi + len(grp), :], pss[:, 0:len(grp), :],
                            AF.Exp)
                        for t, kc in enumerate(grp):
                            d = kc - base
                            mi = {-2: 0, -1: 1, 2: 2, 3: 3}.get(d)
                            if mi is not None:
                                nc.vector.tensor_mul(
                                    pT[:, gi + t, :], pT[:, gi + t, :],
                                    mask_sb[:, mi, :])
                            if kc == 0:
                                nc.vector.memset(pT[0:G, gi + t, :], 0.0)

                    pv = ps_o.tile([128, QB], dt.float32, tag="ps_o", name="ps_o")[:65, :]
                    for j, kc in enumerate(kcs):
                        nc.tensor.matmul(
                            pv, v_sb[:, kc, hi, :], pT[:, j, :],
                            start=(j == 0), stop=False)
                    nc.tensor.matmul(
                        pv, v_sb[0:G, 0, hi, :], pT[0:G, 6, :],
                        start=False, stop=True)
                    lo = G if qb == 0 else 0
                    nc.vector.tensor_copy(
                        ctxT_sb[row:row + 64, hp, q0 + lo:q0 + QB],
                        pv[0:64, lo:])
                    d64 = rbpool.tile([128, QB], dt.float32, tag="rb", name="rb")
                    nc.vector.tensor_copy(d64[64:65, :], pv[64:65, :])
                    nc.sync.dma_start(
                        out=dens[hi * 2 + ql:hi * 2 + ql + 1, :],
                        in_=d64[64:65, :])

        # batched softmax denominators for this chunk
        recip = small.tile([8, QB], dt.float32, tag="recip", name="recip")
        nc.vector.reciprocal(recip, dens)
        drec = dram.tile([8, QB], dt.float32, tag="drec", name="drec")
        nc.sync.dma_start(out=drec, in_=recip)
        for hi in range(4):
            hp, row = hi // 2, (hi % 2) * 64
            for ql in range(2):
                q0 = (scnk * 2 + ql) * QB
                lo = G if (scnk == 0 and ql == 0) else 0
                rb = rbpool.tile([128, QB], dt.float32, tag="rb", name="rb")
                rb = rb[row:row + 64, :]
                src_ap = drec[hi * 2 + ql:hi * 2 + ql + 1, :]
                src_ap = bass.AP(tensor=src_ap.tensor, offset=src_ap.offset,
                                 ap=[[0, 64]] + list(src_ap.ap[1:]))
                nc.gpsimd.dma_start(out=rb, in_=src_ap)
                nc.vector.tensor_mul(
                    ctxT_sb[row:row + 64, hp, q0 + lo:q0 + QB],
                    ctxT_sb[row:row + 64, hp, q0 + lo:q0 + QB],
                    rb[:, lo:])

        # gather ctxT chunk across the 4-core group
        ccol = slice(scnk * 512, scnk * 512 + 512)
        cc_in = dram.tile([256, 512], dt.bfloat16, tag="cc_in", name="cc_in")
        nc.sync.dma_start(
            out=cc_in[:].rearrange("(hp p) s -> p hp s", p=128),
            in_=ctxT_sb[:, :, ccol])
        cc_out = dram.tile([1024, 512], dt.bfloat16, tag="cc_out", name="cc_out")
        nc.gpsimd.collective_compute(
            "AllGather", mybir.AluOpType.bypass,
            replica_groups=[[0, 1, 2, 3], [4, 5, 6, 7]],
            ins=[cc_in[:].opt()], outs=[cc_out[:].opt()])
        # output projection for this chunk: outT[256 E-cols, 512]
        gctx = gtiles.tile([128, 8, 512], dt.bfloat16, tag="gctx", name="gctx")
        nc.sync.dma_start(
            out=gctx, in_=cc_out[:].rearrange("(kt p) s -> p kt s", p=128))
        for mt in range(2):
            pso = ps_big.tile([128, 512], dt.float32, tag="ps_big", name="ps_big")
            for kt in range(8):
                nc.tensor.matmul(
                    pso, wo_sb[:, kt, mt * 128:mt * 128 + 128],
                    gctx[:, kt, :],
                    start=(kt == 0), stop=(kt == 7))
            ot = otile.tile([128, 512], dt.float32, tag="ot", name="ot")
            if scnk == 0:
                nc.vector.tensor_scalar_add(ot[:, 0:G], pso[:, 0:G],
                                            bog_sb[:, mt:mt + 1])
                nc.vector.tensor_scalar_add(ot[:, G:], pso[:, G:],
                                            bow_sb[:, mt:mt + 1])
            else:
                nc.vector.tensor_scalar_add(ot, pso, bow_sb[:, mt:mt + 1])
            c0 = scnk * 512
            nc.sync.dma_start(out=outt[mt, :, c0:c0 + 512], in_=ot)

    ctx.close()


def _host_inputs(x, Wq, bq, Wk, bk, Wv, Wqg, bqg, Wkg, bkg, Wvg, Wo, bo_w, bo_g):
    """Build the 8 per-core input maps."""
    jj = np.arange(128)[:, None]
    ii = np.arange(QB)[None, :]
    masks = np.stack([
        (jj >= ii), (jj >= ii - 128), (jj <= ii), (jj <= ii - 128),
    ]).astype(BF16)

    def b16(a):
        return np.ascontiguousarray(a, dtype=np.float32).astype(BF16)

    xT = [b16(x[b].T) for b in range(B)]
    in_maps = []
    for c in range(8):
        b, hg = c // 4, c % 4
        cs = slice(256 * hg, 256 * hg + 256)
        in_maps.append({
            "xt": xT[b],
            "wq": b16(Wq[:, cs] * 0.125), "wk": b16(Wk[:, cs]),
            "wv": b16(Wv[:, cs]), "wkg": b16(Wkg[:, cs]),
            "wvg": b16(Wvg[:, cs]), "wqg": b16(Wqg[:, cs] * 0.125),
            "wo": b16(Wo[:, cs]),
            "bq": (bq[cs] * 0.125).astype(np.float32),
            "bk": bk[cs].astype(np.float32),
            "bkg": bkg[cs].astype(np.float32),
            "bqg": (bqg[cs] * 0.125).astype(np.float32),
            "bow": bo_w[cs].astype(np.float32),
            "bog": bo_g[cs].astype(np.float32),
            "masks": masks,
        })
    return in_maps


_CACHE = {}


def kernel(hidden_states, key_value_states, Wq, bq, Wk, bk, Wv, bv,
           Wqg, bqg, Wkg, bkg, Wvg, bvg, Wo, bo, num_heads, window,
           num_global, _trace=False):
    x = np.asarray(hidden_states, np.float32)
    args = [np.asarray(a, np.float32) for a in
            (Wq, bq, Wk, bk, Wv, bv, Wqg, bqg, Wkg, bkg, Wvg, bvg, Wo, bo)]
    Wq, bq, Wk, bk, Wv, bv, Wqg, bqg, Wkg, bkg, Wvg, bvg, Wo, bo = args
    bo_w = bo + bv @ Wo
    bo_g = bo + bvg @ Wo

    if "nc" not in _CACHE:
        _CACHE["nc"] = _build_bass()
    nc = _CACHE["nc"]

    in_maps = _host_inputs(x, Wq, bq, Wk, bk, Wv, Wqg, bqg, Wkg, bkg,
                           Wvg, Wo, bo_w, bo_g)
    res = run_bass_kernel_spmd(nc, in_maps, core_ids=list(range(8)),
                               trace=_trace)
    _CACHE["last_result"] = res

    out = np.zeros((B, S, E), np.float32)
    for c in range(8):
        b, hg = c // 4, c % 4
        ot = np.asarray(res.results[c]["outt"], np.float32)  # [2, 128, S]
        out[b, :, 256 * hg:256 * hg + 256] = ot.reshape(256, S).T
    return out

